# revision 6
# baseline (speedup 1.0000x reference)
"""LowRankAttention Trainium2 kernel.

Math shortcut: scores = Q K^T / 8 per (batch, head) has rank <= d_head = 64,
while the truncated SVD keeps rank min(int(1024*0.1), 256) = 102 > 64, so the
low-rank reconstruction is EXACT and the module reduces to plain softmax
attention. Scores are ~N(0,1) (|s| < 8 for these inputs), so exp without
max-subtraction is fp32-safe; the softmax denominator comes for free from a
ones-column appended to the V weights of the PV matmul.

Sharding: 32 (batch, head) pairs over 8 cores; core c owns batch c//4 and
heads 4*(c%4) .. +4 (d_model cols 256*(c%4) .. +256). No collectives.

Pipeline per core (all matmul inputs bf16, fp32 PSUM accumulation):
  stage A: DMA Q,K nat tiles -> PE-transpose -> QT,KT bf16 [128,1024]
           (two heads stacked on partitions 0:64 / 64:128 -> the d=64
           scores matmuls row-pack into the PE array as concurrent pairs)
  kt loop: scores^T matmul (PSUM) -> exp on ScalarE (bf16 out, the
           bottleneck engine) -> PV matmul with [V|1] weights, lagged TWO
           kt steps (crossing group boundaries) so the in-order PE chain
           EXP_i -> PV_i -> scores_{i+1} -> EXP_{i+1} (~1.15us) never
           throttles the ~1.05us ACT exp stream.
  stage C: PE-transpose out^T back to [q, d], reciprocal+scale on DVE, DMA.
Stage A of the next head-pair and stage C of the previous group are emitted
as fillers inside the kt loop to hide them under the exp stream. The final
group's two output stores go on the two HWDGE queues (scalar + sync): a
SWDGE store there would pay ~1us of GpSimd descriptor generation on the
kernel's tail.
"""

import sys

if "/opt/trn_rl_repo" not in sys.path:
    sys.path.insert(0, "/opt/trn_rl_repo")

from collections import deque
from contextlib import ExitStack

import numpy as np

import concourse.bass as bass
import concourse.bacc as bacc
import concourse.tile as tile
from concourse import mybir
from concourse.masks import make_identity
from concourse.bass_utils import run_bass_kernel_spmd

B, S, D = 2, 1024, 1024
H, DH = 16, 64
N_CORES = 8
HPC = 4          # heads per core
CW = HPC * DH    # per-core column width = 256
FP32 = mybir.dt.float32
BF16 = mybir.dt.bfloat16
EXPF = mybir.ActivationFunctionType.Exp
NKT = 8          # k tiles of 128
NQC = 2          # q chunks of 512

_CACHE: dict = {}


def _build_program() -> bass.Bass:
    nc = bacc.Bacc(trn_type="TRN2", num_swdge_queues=4)
    q_d = nc.dram_tensor("q", [S, CW], FP32, kind="ExternalInput")
    k_d = nc.dram_tensor("k", [S, CW], FP32, kind="ExternalInput")
    v_d = nc.dram_tensor("v", [S, CW], FP32, kind="ExternalInput")
    o_d = nc.dram_tensor("o", [S, CW], FP32, kind="ExternalOutput")

    with ExitStack() as ctx:
        tc = ctx.enter_context(tile.TileContext(nc))
        const = ctx.enter_context(tc.tile_pool(name="const", bufs=1))
        nat = ctx.enter_context(tc.tile_pool(name="nat", bufs=5))
        vf_p = ctx.enter_context(tc.tile_pool(name="vf", bufs=2))
        qt_p = ctx.enter_context(tc.tile_pool(name="qt", bufs=2))
        kt_p = ctx.enter_context(tc.tile_pool(name="kt", bufs=2))
        vo_p = ctx.enter_context(tc.tile_pool(name="vo", bufs=4))
        pt_p = ctx.enter_context(tc.tile_pool(name="pt", bufs=4))
        ot_p = ctx.enter_context(tc.tile_pool(name="ot", bufs=4))
        os_p = ctx.enter_context(tc.tile_pool(name="os", bufs=4))
        rc_p = ctx.enter_context(tc.tile_pool(name="rc", bufs=4))
        natb = ctx.enter_context(tc.tile_pool(name="natb", bufs=4))
        idb_p = ctx.enter_context(tc.tile_pool(name="idb", bufs=1))
        # PSUM budget: pss 2 + sc 2x2 + ac 2x1 = 8 banks
        pss = ctx.enter_context(tc.tile_pool(name="pss", bufs=2, space="PSUM"))
        sc_p = ctx.enter_context(tc.tile_pool(name="sc", bufs=2, space="PSUM"))
        ac_p = ctx.enter_context(tc.tile_pool(name="ac", bufs=2, space="PSUM"))

        ident = const.tile([128, 128], FP32)
        make_identity(nc, ident)
        identb = idb_p.tile([128, 128], BF16)
        make_identity(nc, identb)
        # load the exp table set during the prologue DMAs
        warm = const.tile([1, 2], FP32)
        nc.vector.memset(warm, 0.0)
        nc.scalar.activation(out=warm[:, 1:2], in_=warm[:, 0:1], func=EXPF)

        # ---------- emission helpers ----------
        def emit_qk_dma(hp):
            """Batched loads of Q,K nat tiles for head pair hp. For hp0
            (prologue) Q rides the scalar HWDGE queue so both tensors load
            in parallel; hp1 stays on sync (a scalar-queue DMA mid-kernel
            would stall the ACT sequencer feeding the exp stream)."""
            raws = {}
            for name, src in (("k", k_d), ("q", q_d)):
                queue = nc.scalar if (hp == 0 and name == "q") else nc.sync
                halves = []
                for half in range(2):
                    rw = nat.tile([128, 4, 128], FP32, tag="nat",
                                  name=f"{name}raw{hp}_{half}")
                    queue.dma_start(
                        out=rw,
                        in_=src[half * 512:(half + 1) * 512,
                                hp * 128:(hp + 1) * 128].rearrange(
                                    "(t p) c -> p t c", p=128),
                    )
                    halves.append(rw)
                raws[name] = halves
            return raws

    # per-(tensor,half): 4 transposes then one casting copy into t_sb
        def stage_a_steps(hp, raws, bf=False):
            """hp0 (prologue, off the exp stream): fp32 transposes with the
            bf16 cast fused in the PSUM->SBUF copy. hp1 (filler steps inside
            the exp stream): DVE pre-cast to bf16 then 1-cycle/row bf16
            transposes - the PE only has ~75ns/step of slack under the ACT
            exp pacing, so halving the injected transpose work is what keeps
            the stream gapless."""
            tsb = {}
            for name in ("k", "q"):
                pool = kt_p if name == "k" else qt_p
                tsb[name] = pool.tile([128, S], BF16, tag=name + "t",
                                      name=f"{name}t{hp}")
            steps = []
            for half in range(2):
                for name in ("k", "q"):
                    if bf:
                        def cast_step(name=name, half=half):
                            rb = natb.tile([128, 4, 128], BF16, tag="natb",
                                           name=f"nb{hp}{name}{half}")
                            nc.vector.tensor_copy(out=rb,
                                                  in_=raws[name][half])
                            _CACHE[("nb", hp, name, half)] = rb

                        def tp_step(name=name, half=half, t_sb=tsb[name]):
                            rb = _CACHE[("nb", hp, name, half)]
                            ps = pss.tile([128, 512], BF16, tag="pss",
                                          name=f"tps{hp}{name}{half}")
                            for j in range(4):
                                nc.tensor.transpose(
                                    out=ps[:, j * 128:(j + 1) * 128],
                                    in_=rb[:, j, :], identity=identb)
                            dst = t_sb[:, half * 512:(half + 1) * 512]
                            nc.vector.tensor_copy(out=dst, in_=ps)
                        steps.append(cast_step)
                        steps.append(tp_step)
                    else:
                        def tp_step(name=name, half=half, t_sb=tsb[name]):
                            rw = raws[name][half]
                            ps = pss.tile([128, 512], FP32, tag="pss",
                                          name=f"tps{hp}{name}{half}")
                            for j in range(4):
                                nc.tensor.transpose(
                                    out=ps[:, j * 128:(j + 1) * 128],
                                    in_=rw[:, j, :], identity=ident)
                            dst = t_sb[:, half * 512:(half + 1) * 512]
                            # prologue q-half0: ACT is idle, cast there so
                            # it overlaps the DVE copy of k-half0
                            if hp == 0 and half == 0 and name == "q":
                                nc.scalar.copy(out=dst, in_=ps)
                            else:
                                nc.vector.tensor_copy(out=dst, in_=ps)
                        steps.append(tp_step)
            return tsb, steps

        def emit_v_load(hp, hi):
            hcol = (2 * hp + hi) * DH
            vf = vf_p.tile([128, NKT, DH], FP32, tag="vf",
                           name=f"vf{hp}_{hi}")
            nc.sync.dma_start(
                out=vf,
                in_=v_d[:, hcol:hcol + DH].rearrange("(t p) c -> p t c",
                                                     p=128),
            )
            vo = vo_p.tile([128, NKT, DH + 1], BF16, tag="vo",
                           name=f"vo{hp}_{hi}")
            nc.vector.memset(vo[:, :, DH:DH + 1], 1.0)
            # DVE half-casts (the 1.84us Pool cast was finishing barely
            # ahead of its first PV); first 4 kt-blocks land first
            nc.vector.tensor_copy(out=vo[:, 0:4, 0:DH], in_=vf[:, 0:4, :])
            nc.vector.tensor_copy(out=vo[:, 4:NKT, 0:DH], in_=vf[:, 4:NKT, :])
            return vo

        def emit_acc_drain(hp, qc, accs, last=False):
            """PSUM->SBUF copies freeing the accumulator banks, emitted
            eagerly when a group's last PV retires. For the final group ACT
            is idle, so one copy runs there concurrently."""
            for hi in range(2):
                oT = ot_p.tile([DH + 1, 512], FP32, tag="ot",
                               name=f"oT{hp}{qc}{hi}")
                tr = pss.tile([128, 4, DH + 1], FP32, tag="pss",
                              name=f"tr{hp}{qc}{hi}")
                _CACHE[("c", hp, qc, hi)] = (oT, tr)
                if last and hi == 0:
                    nc.scalar.copy(out=oT, in_=accs[hi])
                else:
                    nc.vector.tensor_copy(out=oT, in_=accs[hi])

        def stage_c_steps(hp, qc, last=False):

            def c_tp(hi, lo, hicnt):
                # split in 2-transpose closures: a 4-transpose step at a
                # group boundary costs the PE its one-step lead on the ACT
                # exp stream (~285ns gap per boundary)
                oT, tr = _CACHE[("c", hp, qc, hi)]
                for qt in range(lo, lo + hicnt):
                    nc.tensor.transpose(
                        out=tr[:, qt, :],
                        in_=oT[:, qt * 128:(qt + 1) * 128],
                        identity=ident[0:DH + 1, 0:DH + 1])

            def c_norm(hi):
                oT, tr = _CACHE[("c", hp, qc, hi)]
                r4 = rc_p.tile([128, 4], FP32, tag="rc",
                               name=f"r4{hp}{qc}{hi}")
                nc.vector.reciprocal(out=r4, in_=tr[:, :, DH:DH + 1])
                osb = os_p.tile([128, 4, DH], FP32, tag="os",
                                name=f"os{hp}{qc}{hi}")
                r4b = bass.AP(tensor=r4.tensor, offset=r4.offset,
                              ap=[r4.ap[0], [1, 4], [0, DH]])
                nc.vector.tensor_tensor(
                    out=osb, in0=tr[:, :, 0:DH], in1=r4b,
                    op=mybir.AluOpType.mult)
                hcol = (2 * hp + hi) * DH
                if last:
                    # tail: both queues HWDGE, no SWDGE descgen latency
                    eng = nc.scalar if hi == 0 else nc.sync
                else:
                    eng = nc.gpsimd if hi == 0 else nc.sync
                eng.dma_start(
                    out=o_d[qc * 512:(qc + 1) * 512,
                            hcol:hcol + DH].rearrange(
                                "(t p) c -> p t c", p=128),
                    in_=osb,
                )
            return [lambda: c_tp(0, 0, 2), lambda: c_tp(0, 2, 2),
                    lambda: c_norm(0),
                    lambda: c_tp(1, 0, 2), lambda: c_tp(1, 2, 2),
                    lambda: c_norm(1)]

        # ---------- prologue ----------
        raws0 = emit_qk_dma(0)
        tsb0, stA0 = stage_a_steps(0, raws0)
        for st in stA0:
            st()
        vos0 = [emit_v_load(0, 0), emit_v_load(0, 1)]

        fillers: deque = deque()
        fillers.append(lambda: _CACHE.__setitem__("raws1", emit_qk_dma(1)))
        fillers.append(lambda: vos1.append(emit_v_load(1, 0)))
        fillers.append(lambda: vos1.append(emit_v_load(1, 1)))
        vos1: list = []
        tsb1: dict = {}

        def queue_stage_a1():
            t, steps = stage_a_steps(1, _CACHE["raws1"], bf=True)
            tsb1.update(t)
            return steps

        # flat 32-step pipeline: PV trails the exp stream by TWO steps and
        # crosses group boundaries. With lag-1 the pacing loop was
        # EXP_i -> (sem) -> PV_i -> sc_{i+1} -> (sem) -> EXP_{i+1} on the
        # in-order PE (~1.15us > the 1.09us exp), throttling the ACT
        # stream; at lag 2 the PE always runs a full step ahead.
        groups = [(0, 0), (0, 1), (1, 0), (1, 1)]
        stA1_queued = False
        pend = deque()  # (pt, kt, accs, vos, hp, qc, last)

        def flush_pv(entry):
            ppt, pkt, accs_, vos_, hp_, qc_, last_ = entry
            for hi in range(2):
                nc.tensor.matmul(
                    accs_[hi],
                    lhsT=vos_[hi][:, pkt, :],
                    rhs=ppt[:, hi * 512:(hi + 1) * 512],
                    start=(pkt == 0), stop=(pkt == NKT - 1),
                )
            if pkt == NKT - 1:
                emit_acc_drain(hp_, qc_, accs_, last=last_)
                fillers.extend(stage_c_steps(hp_, qc_, last=last_))

        for gi, (hp, qc) in enumerate(groups):
            qt_sb = (tsb0 if hp == 0 else tsb1)["q"]
            kt_sb = (tsb0 if hp == 0 else tsb1)["k"]
            vos = vos0 if hp == 0 else vos1
            accs = [ac_p.tile([DH + 1, 512], FP32, tag="ac",
                              name=f"acc{hp}_{qc}_{i}") for i in range(2)]
            for kt in range(NKT):
                sc = sc_p.tile([128, 1024], FP32, tag="sc",
                               name=f"sc{gi}_{kt}")
                for hi in range(2):
                    nc.tensor.matmul(
                        sc[:, hi * 512:(hi + 1) * 512],
                        lhsT=kt_sb[hi * 64:(hi + 1) * 64,
                                   kt * 128:(kt + 1) * 128],
                        rhs=qt_sb[hi * 64:(hi + 1) * 64,
                                  qc * 512:(qc + 1) * 512],
                        start=True, stop=True,
                    )
                while len(pend) >= 2:
                    flush_pv(pend.popleft())
                # filler work (next stage A / prev stage C / V loads)
                n_pop = 2 if (gi == 0 and kt == 0) else 1
                for _ in range(n_pop):
                    if fillers:
                        fillers.popleft()()
                if gi == 0 and kt == 2 and not stA1_queued:
                    fillers.extend(queue_stage_a1())
                    stA1_queued = True
                pt = pt_p.tile([128, 1024], BF16, tag="pt",
                               name=f"pt{gi}_{kt}")
                nc.scalar.activation(out=pt, in_=sc, func=EXPF, scale=0.125)
                pend.append((pt, kt, accs, vos, hp, qc,
                             gi == len(groups) - 1))

        while pend:
            flush_pv(pend.popleft())
        while fillers:
            fillers.popleft()()

    if not nc.is_finalized():
        nc.finalize()
    return nc


def kernel(query: np.ndarray, key: np.ndarray, value: np.ndarray,
           _trace: bool = False):
    if "nc" not in _CACHE:
        _CACHE["nc"] = _build_program()
    nc = _CACHE["nc"]

    query = np.ascontiguousarray(query, dtype=np.float32)
    key = np.ascontiguousarray(key, dtype=np.float32)
    value = np.ascontiguousarray(value, dtype=np.float32)

    in_maps = []
    for c in range(N_CORES):
        b, g = divmod(c, HPC)
        cols = slice(g * CW, (g + 1) * CW)
        in_maps.append({
            "q": np.ascontiguousarray(query[b, :, cols]),
            "k": np.ascontiguousarray(key[b, :, cols]),
            "v": np.ascontiguousarray(value[b, :, cols]),
        })

    res = run_bass_kernel_spmd(
        nc, in_maps, core_ids=list(range(N_CORES)), trace=_trace
    )
    out = np.empty((B, S, D), dtype=np.float32)
    for c in range(N_CORES):
        b, g = divmod(c, HPC)
        out[b, :, g * CW:(g + 1) * CW] = res.results[c]["o"]
    if _trace:
        _CACHE["last_result"] = res
    return out

